# revision 1
# baseline (speedup 1.0000x reference)
"""DiarizeGPT Trainium2 kernel — data-parallel over batch (1 sample per NeuronCore, 8 cores).

Device dataflow per core (sample b):
  stage1: audio projection (PE, bf16) + packing (mask-mult + host-prepared dense add-in)
  8 decoder layers, canonical activation layout x[S=1152 (9 p-chunks), D=768] fp32 in SBUF:
    LN (bn_stats) -> h bf16 -> PE-transpose -> hT [768, 1152]
    qkv (bf16 weights), per-head causal attention with transposed scores [k, q],
    softmax denominator fused into the AV matmul via a ones-column on v (fp32r),
    attn_o + residual, MLP (fc -> exact gelu -> mlp_o) in q-blocks with PSUM-resident
    accumulation, + residual.
  final LN -> one-hot gather matmul (lat) -> head matmul -> log-softmax CE -> loss[90]
Host: shard/prep inputs (index/mask/one-hot artifacts + weight transposes, zero FLOPs),
 sum the 8x90 masked token losses and divide by the non-pad count.
"""

import numpy as np
import ml_dtypes

import concourse.bass as bass
import concourse.mybir as mybir
import concourse.tile as tile
from concourse import bacc
from concourse.bass_utils import run_bass_kernel_spmd

F32 = mybir.dt.float32
F32R = mybir.dt.float32r
BF16 = mybir.dt.bfloat16
AF = mybir.ActivationFunctionType
OP = mybir.AluOpType

P = 128
D = 768; H = 12; NL = 8; V = 128
LATENT = 1024; TA = 1000; L = 90; S = 1120; B = 8
SP = 1152          # padded sequence (9 * 128)
NSC = SP // P      # 9 sequence chunks
NDC = D // P       # 6 feature chunks
NLC = LATENT // P  # 8 latent chunks
DM = 4 * D         # 3072 mlp hidden
NMC = DM // P      # 24
DH = 64
NST = 3            # q strips of 384
ST = 384

_NC_CACHE = {}


def build_nc(debug_taps=False, n_layers=NL, sim_gelu_identity=False):
    nc = bacc.Bacc("TRN2", target_bir_lowering=False, debug=False)

    # ---- DRAM inputs (per-core data; weights identical across cores) ----
    d_audioT = nc.dram_tensor("audioT", [NLC, P, 1024], BF16, kind="ExternalInput")
    d_wp = nc.dram_tensor("wp", [NLC, P, D], BF16, kind="ExternalInput")
    d_addin = nc.dram_tensor("addin", [NSC, P, D], F32, kind="ExternalInput")
    d_maskA = nc.dram_tensor("maskA", [NSC, P], F32, kind="ExternalInput")
    d_wqk = nc.dram_tensor("wqk", [NL, NDC, P, 2 * D], BF16, kind="ExternalInput")
    d_wv = nc.dram_tensor("wv", [NL, NDC, P, D], BF16, kind="ExternalInput")
    d_wo = nc.dram_tensor("wo", [NL, NDC, P, D], BF16, kind="ExternalInput")
    d_fc = nc.dram_tensor("fc", [NL, NMC // 2, NDC, P, 2 * P], BF16, kind="ExternalInput")
    d_mo = nc.dram_tensor("mo", [NL, NMC, P, D], BF16, kind="ExternalInput")
    d_head = nc.dram_tensor("head", [NDC, P, V], F32, kind="ExternalInput")
    d_G = nc.dram_tensor("G", [NSC, P, L], F32, kind="ExternalInput")
    d_wsel = nc.dram_tensor("wsel", [L, V], F32, kind="ExternalInput")
    d_mrow = nc.dram_tensor("mrow", [L, 1], F32, kind="ExternalInput")
    d_loss = nc.dram_tensor("loss", [L, 1], F32, kind="ExternalOutput")

    taps = {}
    if debug_taps:
        taps["x0"] = nc.dram_tensor("tap_x0", [NSC, P, D], F32, kind="ExternalOutput")
        for l in range(n_layers):
            taps[f"x{l + 1}"] = nc.dram_tensor(f"tap_x{l + 1}", [NSC, P, D], F32,
                                               kind="ExternalOutput")
        taps["logits"] = nc.dram_tensor("tap_logits", [L, V], F32, kind="ExternalOutput")

    # inline constants (baked into NEFF, loaded at model-load time)
    ident_np = np.eye(P, dtype=ml_dtypes.bfloat16)
    d_ident = nc.inline_tensor(ident_np, name="identity")
    minv_np = np.zeros((P, 640), dtype=np.uint8)
    for r in range(P):
        minv_np[r, : min(512 + r, 640)] = 1.0  # 1 where INVALID (k > q)
    d_minv = nc.inline_tensor(minv_np, name="minv")

    with tile.TileContext(nc) as tc:
        with tc.tile_pool(name="singles", bufs=1) as singles, \
             tc.tile_pool(name="persist", bufs=1) as persist, \
             tc.tile_pool(name="wpool", bufs=1) as wpool, \
             tc.tile_pool(name="hT", bufs=1) as hTp, \
             tc.tile_pool(name="big", bufs=1) as bigp, \
             tc.tile_pool(name="str4", bufs=3) as str4, \
             tc.tile_pool(name="str3", bufs=3) as str3, \
             tc.tile_pool(name="denp", bufs=2) as denp, \
             tc.tile_pool(name="denrepp", bufs=2) as denrepp, \
             tc.tile_pool(name="sexpp", bufs=3) as sexpp, \
             tc.tile_pool(name="stats", bufs=6) as statsp, \
             tc.tile_pool(name="ps384", bufs=4, space="PSUM") as ps384, \
             tc.tile_pool(name="pout", bufs=2, space="PSUM") as poutp, \
             tc.tile_pool(name="dramp", bufs=2, space="DRAM") as dramp:

            # ---- singles ----
            ident_sb = singles.tile([P, P], BF16, tag="ident")
            nc.sync.dma_start(ident_sb, d_ident[:, :])
            minv_sb = singles.tile([P, 640], mybir.dt.uint8, tag="minv")
            nc.sync.dma_start(minv_sb, d_minv[:, :])
            zeros_sb = singles.tile([P, ST], BF16, tag="zeros")
            nc.vector.memset(zeros_sb, 0.0)
            eps_sb = singles.tile([P, 1], F32, tag="eps")
            nc.vector.memset(eps_sb, 1e-5)
            bias0_sb = singles.tile([P, 1], F32, tag="bias0")
            nc.vector.memset(bias0_sb, 0.0)
            maskA_sb = singles.tile([P, NSC], F32, tag="maskA")
            nc.sync.dma_start(maskA_sb, d_maskA[:, :].rearrange("c p -> p c"))
            G_sb = singles.tile([P, NSC, L], F32, tag="G")
            nc.sync.dma_start(G_sb, d_G[:, :, :].rearrange("c p j -> p c j"))
            head_sb = singles.tile([P, NDC, V], F32, tag="head")
            nc.sync.dma_start(head_sb, d_head[:, :, :].rearrange("c p v -> p c v"))
            wsel_sb = singles.tile([L, V], F32, tag="wsel")
            nc.sync.dma_start(wsel_sb, d_wsel[:, :])
            mrow_sb = singles.tile([L, 1], F32, tag="mrow")
            nc.sync.dma_start(mrow_sb, d_mrow[:, :])

            # persistent residual stream
            x_sb = persist.tile([P, NSC, D], F32, tag="x")

            # ================= stage 1: audio projection + packing =================
            with tc.tile_pool(name="stage1", bufs=1) as s1p, \
                 tc.tile_pool(name="addin", bufs=2) as adp, \
                 tc.tile_pool(name="s1str", bufs=3) as s1str:
                wp_sb = s1p.tile([P, NLC, D], BF16, tag="wp")
                nc.sync.dma_start(wp_sb, d_wp[:, :, :].rearrange("c p d -> p c d"))

                for sc in range(8):  # s rows 0..1023 get the audio matmul
                    at_sb = s1str.tile([P, NLC, P], BF16, tag="audioT")
                    nc.sync.dma_start(
                        at_sb, d_audioT[:, :, sc * P:(sc + 1) * P].rearrange("c p s -> p c s"))
                    ad = adp.tile([P, D], F32, tag="addin")
                    nc.sync.dma_start(ad, d_addin[sc, :, :])
                    for half in range(2):
                        cols = slice(half * ST, half * ST + ST)
                        pm = ps384.tile([P, ST], F32, tag="p384")
                        for lc in range(NLC):
                            nc.tensor.matmul(
                                pm, at_sb[:, lc, :],
                                wp_sb[:, lc, cols],
                                start=(lc == 0), stop=(lc == NLC - 1))
                        nc.vector.tensor_scalar_mul(
                            x_sb[:, sc, cols], pm, maskA_sb[:, sc:sc + 1])
                        nc.vector.tensor_add(
                            x_sb[:, sc, cols], x_sb[:, sc, cols], ad[:, cols])
                # pad chunk: pure add-in (start token / text can't reach past 1090)
                nc.sync.dma_start(x_sb[:, 8, :], d_addin[8, :, :])

            if debug_taps:
                nc.sync.dma_start(taps["x0"][:, :, :].rearrange("c p d -> p c d"), x_sb)

            # ================= decoder layers =================
            def layer_norm_to_hT(l, gi, hT_sb):
                """LN of x (free-dim stats) -> bf16 chunks -> PE transpose -> hT [p, dc, SP]."""
                for sc in range(NSC):
                    st_t = statsp.tile([P, 3, 6], F32, tag="bnst")
                    for g in range(3):
                        nc.vector.bn_stats(st_t[:, g, :], x_sb[:, sc, g * 256:(g + 1) * 256])
                    mv = statsp.tile([P, 2], F32, tag="bnmv")
                    nc.vector.bn_aggr(mv, st_t)
                    rstd = statsp.tile([P, 1], F32, tag="rstd")
                    nc.scalar.activation(rstd, mv[:, 1:2], AF.Sqrt, bias=eps_sb, scale=1.0)
                    nc.vector.reciprocal(rstd, rstd)
                    hc = str3.tile([P, D], BF16, tag="hc")
                    nc.vector.tensor_scalar(hc, x_sb[:, sc, :], mv[:, 0:1], rstd,
                                            OP.subtract, OP.mult)
                    for dc in range(NDC):
                        ptt = ps384.tile([P, ST], F32, tag="p384")
                        pt = ptt[:, :].bitcast(BF16)[:, 0:P]
                        nc.tensor.transpose(pt, hc[:, dc * P:(dc + 1) * P], ident_sb)
                        dst = hT_sb[:, dc, sc * P:(sc + 1) * P]
                        if dc % 2 == 0:
                            nc.vector.tensor_copy(dst, pt)
                        else:
                            nc.scalar.copy(dst, pt)

            for l in range(n_layers):
                # ---- layer weights (resident for the layer) ----
                wv_sb = wpool.tile([P, NDC, D], BF16, tag="wv")
                nc.sync.dma_start(wv_sb, d_wv[l].rearrange("c p n -> p c n"))
                wo_sb = wpool.tile([P, NDC, D], BF16, tag="wo")
                nc.sync.dma_start(wo_sb, d_wo[l].rearrange("c p n -> p c n"))

                # ---- LN1 + transpose ----
                hT_sb = hTp.tile([P, NDC, SP], BF16, tag="hT")
                layer_norm_to_hT(l, 0, hT_sb)

                # ---- q/k (transposed layout) ----
                qkT_sb = bigp.tile([P, H, SP], BF16, tag="qkT")
                for oc in range(H):  # 12 chunks of 128 rows: q = oc 0..5, k = oc 6..11
                    wqkb = str3.tile([P, NDC, P], BF16, tag="wqkb")
                    nc.sync.dma_start(
                        wqkb, d_wqk[l, :, :, oc * P:(oc + 1) * P].rearrange("c p n -> p c n"))
                    for ns in range(NST):
                        cols = slice(ns * ST, ns * ST + ST)
                        pm = ps384.tile([P, ST], F32, tag="p384")
                        for dc in range(NDC):
                            nc.tensor.matmul(
                                pm, wqkb[:, dc, :],
                                hT_sb[:, dc, cols],
                                start=(dc == 0), stop=(dc == NDC - 1))
                        nc.vector.tensor_copy(qkT_sb[:, oc, cols], pm)

                # ---- v (standard layout, 65-wide head blocks w/ ones column) ----
                v65_sb = bigp.tile([P, NSC, H, 65], BF16, tag="v65")
                for sc in range(NSC):
                    for half in range(2):
                        pm = ps384.tile([P, ST], F32, tag="p384")
                        for dc in range(NDC):
                            nc.tensor.matmul(
                                pm, hT_sb[:, dc, sc * P:(sc + 1) * P],
                                wv_sb[:, dc, half * ST:half * ST + ST],
                                start=(dc == 0), stop=(dc == NDC - 1))
                        nc.vector.tensor_copy(
                            v65_sb[:, sc, half * 6:(half + 1) * 6, 0:DH],
                            pm.rearrange("p (h e) -> p h e", e=DH))
                    nc.gpsimd.memset(v65_sb[:, sc, :, DH:65], 1.0)

                # ---- attention ----
                oT_sb = bigp.tile([P, NDC, SP], BF16, tag="oT")
                for hp in range(6):
                    den_h = [denp.tile([1, SP], F32, tag="denh", name=f"denh{_i}")
                             for _i in range(2)]
                    for st in range(NST):
                        K = 3 * (st + 1)
                        pav = [ps384.tile([P, ST], F32, tag="p384", name=f"pav{_i}")[0:65, :] for _i in range(2)]
                        se_prev = [None, None]
                        for ki in range(K):
                            o = ki * P - st * ST
                            q0 = max(0, o)
                            se_cur = [None, None]
                            for hh in range(2):
                                h = 2 * hp + hh
                                rows = slice(DH * hh, DH * hh + DH)
                                ps = ps384.tile([P, ST], F32, tag="p384")
                                nc.tensor.matmul(
                                    ps[:, q0:ST],
                                    qkT_sb[rows, 6 + hp, ki * P:(ki + 1) * P],
                                    qkT_sb[rows, hp, st * ST + q0:(st + 1) * ST],
                                    start=True, stop=True)
                                se = sexpp.tile([P, ST], BF16, tag="sexp")
                                nc.scalar.activation(se[:, q0:ST], ps[:, q0:ST],
                                                     AF.Exp, bias=bias0_sb, scale=0.125)
                                if o >= 0:
                                    nc.vector.copy_predicated(
                                        se[:, 0:o + P],
                                        minv_sb[:, 512 - o:640],
                                        zeros_sb[:, 0:o + P])
                                se_cur[hh] = se
                            # software pipeline: issue prev ki's AV after this ki's scores
                            for hh in range(2):
                                if ki > 0:
                                    nc.tensor.matmul(
                                        pav[hh],
                                        v65_sb[:, ki - 1, 2 * hp + hh, :],
                                        se_prev[hh][:, :],
                                        start=(ki == 1), stop=False)
                                se_prev[hh] = se_cur[hh]
                        for hh in range(2):
                            nc.tensor.matmul(
                                pav[hh],
                                v65_sb[:, K - 1, 2 * hp + hh, :],
                                se_prev[hh][:, :],
                                start=(K == 1), stop=True)
                        for hh in range(2):
                            h = 2 * hp + hh
                            cols = slice(st * ST, st * ST + ST)
                            nc.vector.tensor_copy(
                                oT_sb[DH * hh:DH * hh + DH, hp, cols], pav[hh][0:DH, :])
                            nc.scalar.copy(den_h[hh][0:1, cols], pav[hh][DH:DH + 1, :])
                    # normalize this head-pair's output chunk by 1/den
                    dden = dramp.tile([2, SP], BF16, tag="dden")
                    for hh in range(2):
                        denr = denp.tile([1, SP], F32, tag="denr")
                        nc.vector.reciprocal(denr, den_h[hh])
                        denrb = denp.tile([1, SP], BF16, tag="denrb")
                        nc.vector.tensor_copy(denrb, denr)
                        nc.sync.dma_start(dden[hh:hh + 1, :], denrb)
                    denrep = denrepp.tile([P, SP], BF16, tag="denrep")
                    for hh in range(2):
                        row = dden[hh:hh + 1, :]
                        bc = bass.AP(tensor=row.tensor, offset=row.offset,
                                     ap=[[0, DH], list(row.ap[1])])
                        nc.sync.dma_start(denrep[DH * hh:DH * hh + DH, :], bc)
                    nc.vector.tensor_tensor(oT_sb[:, hp, :], oT_sb[:, hp, :], denrep,
                                            OP.mult)

                # ---- attn_o + residual ----
                for sc in range(NSC):
                    for half in range(2):
                        cols = slice(half * ST, half * ST + ST)
                        pm = ps384.tile([P, ST], F32, tag="p384")
                        for dc in range(NDC):
                            nc.tensor.matmul(
                                pm, oT_sb[:, dc, sc * P:(sc + 1) * P],
                                wo_sb[:, dc, cols],
                                start=(dc == 0), stop=(dc == NDC - 1))
                        nc.vector.tensor_add(x_sb[:, sc, cols], x_sb[:, sc, cols], pm)

                # ---- LN2 + transpose ----
                h2T_sb = hTp.tile([P, NDC, SP], BF16, tag="hT")
                layer_norm_to_hT(l, 1, h2T_sb)

                # ---- MLP: fc -> gelu -> mlp_o, q-blocks of 2 chunks ----
                for qb0, qbn in ((0, 2), (2, 2), (4, 2), (6, 2), (8, 1)):
                    qcols = slice(qb0 * P, (qb0 + qbn) * P)
                    qw = qbn * P
                    pA = [poutp.tile([P, 512], F32, tag="poutA", name=f"pA{_i}")
                          for _i in range(qbn)]
                    pB = [poutp.tile([P, 256], F32, tag="poutB", name=f"pB{_i}")
                          for _i in range(qbn)]
                    hid_prev = None
                    for dm in range(NMC):
                        if dm % 2 == 0:
                            fcb_cur = str4.tile([P, NDC, 2 * P], BF16, tag="fcb")
                            nc.sync.dma_start(fcb_cur,
                                              d_fc[l, dm // 2].rearrange("c p n -> p c n"))
                        mob = str4.tile([P, D], BF16, tag="mob")
                        nc.sync.dma_start(mob, d_mo[l, dm])
                        ph = ps384.tile([P, ST], F32, tag="p384")
                        koff = (dm % 2) * P
                        for dc in range(NDC):
                            nc.tensor.matmul(
                                ph[:, 0:qw], fcb_cur[:, dc, koff:koff + P],
                                h2T_sb[:, dc, qcols],
                                start=(dc == 0), stop=(dc == NDC - 1))
                        hid = str3.tile([P, ST], BF16, tag="hid")
                        nc.scalar.activation(hid[:, 0:qw], ph[:, 0:qw],
                                             AF.Identity if sim_gelu_identity else AF.Gelu,
                                             bias=bias0_sb)
                        # pipeline: previous dm's mlp_o after this dm's fc
                        if hid_prev is not None:
                            for qc in range(qbn):
                                lh = hid_prev[0][:, qc * P:(qc + 1) * P]
                                nc.tensor.matmul(pA[qc], lh, hid_prev[1][:, 0:512],
                                                 start=(dm == 1), stop=False)
                                nc.tensor.matmul(pB[qc], lh, hid_prev[1][:, 512:D],
                                                 start=(dm == 1), stop=False)
                        hid_prev = (hid, mob)
                    for qc in range(qbn):
                        lh = hid_prev[0][:, qc * P:(qc + 1) * P]
                        nc.tensor.matmul(pA[qc], lh, hid_prev[1][:, 0:512],
                                         start=False, stop=True)
                        nc.tensor.matmul(pB[qc], lh, hid_prev[1][:, 512:D],
                                         start=False, stop=True)
                    for qc in range(qbn):
                        sc = qb0 + qc
                        nc.vector.tensor_add(x_sb[:, sc, 0:512], x_sb[:, sc, 0:512], pA[qc])
                        nc.vector.tensor_add(x_sb[:, sc, 512:D], x_sb[:, sc, 512:D], pB[qc])

                if debug_taps:
                    nc.sync.dma_start(
                        taps[f"x{l + 1}"][:, :, :].rearrange("c p d -> p c d"), x_sb)

            # ================= final LN (in place) + gather + head + CE =================
            for sc in range(NSC):
                st_t = statsp.tile([P, 3, 6], F32, tag="bnst")
                for g in range(3):
                    nc.vector.bn_stats(st_t[:, g, :], x_sb[:, sc, g * 256:(g + 1) * 256])
                mv = statsp.tile([P, 2], F32, tag="bnmv")
                nc.vector.bn_aggr(mv, st_t)
                rstd = statsp.tile([P, 1], F32, tag="rstd")
                nc.scalar.activation(rstd, mv[:, 1:2], AF.Sqrt, bias=eps_sb, scale=1.0)
                nc.vector.reciprocal(rstd, rstd)
                nc.vector.tensor_scalar(x_sb[:, sc, :], x_sb[:, sc, :], mv[:, 0:1], rstd,
                                        OP.subtract, OP.mult)

            latT_sb = singles.tile([P, NDC, L], F32, tag="latT")
            for dc in range(NDC):
                pl = ps384.tile([P, ST], F32, tag="p384", name="plat")[:, 0:L]
                for sc in range(NSC):
                    nc.tensor.matmul(
                        pl, x_sb[:, sc, dc * P:(dc + 1) * P],
                        G_sb[:, sc, :],
                        start=(sc == 0), stop=(sc == NSC - 1))
                nc.vector.tensor_copy(latT_sb[:, dc, :], pl)

            plog = ps384.tile([P, ST], F32, tag="p384", name="plog")[0:L, 0:V]
            for dc in range(NDC):
                nc.tensor.matmul(plog, latT_sb[:, dc, :],
                                 head_sb[:, dc, :],
                                 start=(dc == 0), stop=(dc == NDC - 1))
            lg = singles.tile([L, V], F32, tag="lg")
            nc.vector.tensor_copy(lg, plog)
            if debug_taps:
                nc.sync.dma_start(taps["logits"][:, :], lg)

            m1 = singles.tile([L, 1], F32, tag="m1")
            nc.vector.reduce_max(m1, lg, axis=mybir.AxisListType.X)
            negm = singles.tile([L, 1], F32, tag="negm")
            nc.vector.tensor_scalar_mul(negm, m1, -1.0)
            e_sb = singles.tile([L, V], F32, tag="e")
            s1 = singles.tile([L, 1], F32, tag="s1")
            nc.scalar.activation(e_sb, lg, AF.Exp, bias=negm, scale=1.0, accum_out=s1)
            lse = singles.tile([L, 1], F32, tag="lse")
            nc.scalar.activation(lse, s1, AF.Ln, bias=bias0_sb[0:L, :])
            junk = singles.tile([L, V], F32, tag="junk")
            tgt = singles.tile([L, 1], F32, tag="tgt")
            nc.vector.tensor_tensor(junk, lg, wsel_sb, OP.mult)
            nc.vector.reduce_sum(tgt, junk, axis=mybir.AxisListType.X)
            t1 = singles.tile([L, 1], F32, tag="t1")
            nc.vector.tensor_add(t1, m1, lse)
            nc.vector.tensor_tensor(t1, t1, tgt, OP.subtract)
            nc.vector.tensor_tensor(t1, t1, mrow_sb, OP.mult)
            nc.sync.dma_start(d_loss[:, :], t1)

    nc.compile()
    return nc


# ======================= host side =======================

def _sinu_pos(n, dim):
    half = dim // 2
    inv_freq = np.float32(10000.0) ** (-(np.arange(half, dtype=np.float32) / np.float32(half)))
    ang = np.arange(n, dtype=np.float32)[:, None] * inv_freq[None, :]
    return np.concatenate([np.sin(ang), np.cos(ang)], axis=-1).astype(np.float32)


def prep_shared(inp):
    """Weight-layout prep, shared across cores."""
    f32 = lambda a: np.asarray(a, dtype=np.float32)
    qkv_w = f32(inp["qkv_w"]); fc_w = f32(inp["fc_w"])
    attn_o_w = f32(inp["attn_o_w"]); mlp_o_w = f32(inp["mlp_o_w"])
    ln1_g = f32(inp["ln1_g"]); ln2_g = f32(inp["ln2_g"]); lnf_g = f32(inp["lnf_g"])
    bf = ml_dtypes.bfloat16

    wqk = np.empty((NL, NDC, P, 2 * D), dtype=bf)
    wv = np.empty((NL, NDC, P, D), dtype=bf)
    wo = np.empty((NL, NDC, P, D), dtype=bf)
    fcb = np.empty((NL, NMC // 2, NDC, P, 2 * P), dtype=bf)
    mo = np.empty((NL, NMC, P, D), dtype=bf)
    for l in range(NL):
        wqk_eff = (qkv_w[l, :2 * D] * ln1_g[l][None, :]).T          # [768, 1536]
        wv_eff = (qkv_w[l, 2 * D:] * ln1_g[l][None, :]).T           # [768, 768]
        wqk[l] = wqk_eff.reshape(NDC, P, 2 * D).astype(bf)
        wv[l] = wv_eff.reshape(NDC, P, D).astype(bf)
        wo[l] = attn_o_w[l].T.reshape(NDC, P, D).astype(bf)
        fc_eff = (fc_w[l] * ln2_g[l][None, :]).T                    # [768, 3072]
        fcb[l] = fc_eff.reshape(NDC, P, NMC // 2, 2 * P).transpose(2, 0, 1, 3).astype(bf)
        mo[l] = mlp_o_w[l].T.reshape(NMC, P, D).astype(bf)

    head = (f32(inp["text_head_w"]) * lnf_g[None, :]).T.reshape(NDC, P, V).copy()
    wp = f32(inp["audio_proj_w"]).T.reshape(NLC, P, D).astype(bf)

    posb_a = np.zeros((1024, D), dtype=np.float32)
    posb_a[:TA] = _sinu_pos(TA, D) * np.float32(inp["audio_pos_scale"]) \
        + f32(inp["audio_proj_b"])[None, :]
    pos_t = _sinu_pos(L, D) * np.float32(inp["text_pos_scale"])
    return dict(wqk=wqk, wv=wv, wo=wo, fc=fcb, mo=mo, head=head, wp=wp,
                posb_a=posb_a, pos_t=pos_t)


def prep_core(inp, shared, b):
    f32 = lambda a: np.asarray(a, dtype=np.float32)
    bf = ml_dtypes.bfloat16
    labels = np.asarray(inp["labels"]).astype(np.int64)[b]          # [90]
    al = int(np.asarray(inp["audio_lengths"]).astype(np.int64)[b])
    ll = int(np.asarray(inp["label_lengths"]).astype(np.int64)[b])
    text_emb_w = f32(inp["text_emb_w"])
    start_emb = f32(inp["start_emb"])

    audioT = np.zeros((LATENT, 1024), dtype=bf)
    audioT[:, :TA] = f32(inp["audio"][b]).T.astype(bf)

    maskA = np.zeros(SP, dtype=np.float32)
    maskA[:al] = 1.0

    addin = np.zeros((SP, D), dtype=np.float32)
    addin[:1024] = shared["posb_a"] * maskA[:1024, None]
    addin[al] = start_emb
    ntxt = ll - 1  # text tokens used as input (shifted right by start)
    if ntxt > 0:
        addin[al + 1: al + 1 + ntxt] = text_emb_w[labels[:ntxt]] + shared["pos_t"][:ntxt]

    G = np.zeros((SP, L), dtype=np.float32)
    G[al + np.arange(L), np.arange(L)] = 1.0

    wsel = np.zeros((L, V), dtype=np.float32)
    valid = labels != 0
    wsel[np.arange(L)[valid], labels[valid]] = 1.0
    mrow = valid.astype(np.float32).reshape(L, 1)

    return {
        "audioT": audioT.reshape(NLC, P, 1024),
        "addin": addin.reshape(NSC, P, D),
        "maskA": maskA.reshape(NSC, P),
        "G": G.reshape(NSC, P, L),
        "wsel": wsel, "mrow": mrow,
        "wp": shared["wp"], "wqk": shared["wqk"], "wv": shared["wv"],
        "wo": shared["wo"], "fc": shared["fc"], "mo": shared["mo"],
        "head": shared["head"],
    }


def get_nc(debug_taps=False, n_layers=NL):
    key = (bool(debug_taps), n_layers)
    if key not in _NC_CACHE:
        _NC_CACHE[key] = build_nc(debug_taps=key[0], n_layers=n_layers)
    return _NC_CACHE[key]


def run_cores(inputs, debug_taps=False, trace=False, n_layers=NL):
    nc = get_nc(debug_taps=debug_taps, n_layers=n_layers)
    shared = prep_shared(inputs)
    in_maps = [prep_core(inputs, shared, b) for b in range(B)]
    res = run_bass_kernel_spmd(nc, in_maps, core_ids=list(range(B)), trace=trace)
    return res


def kernel(**inputs) -> np.ndarray:
    res = run_cores(inputs)
    labels = np.asarray(inputs["labels"]).astype(np.int64)
    count = np.count_nonzero(labels)
    total = sum(float(r["loss"].sum()) for r in res.results)
    return np.float32(total / count)


def bench(inputs, iters=5):
    """Steady-state device execution timing: stage inputs on-device once, then
    time repeated jitted executions (mirrors bass2jax.run_bass_via_pjrt)."""
    import time
    import jax
    from jax.sharding import Mesh, PartitionSpec, NamedSharding
    from jax.experimental.shard_map import shard_map
    from concourse import mybir as _mybir
    from concourse.bass2jax import (_bass_exec_p, install_neuronx_cc_hook,
                                    partition_id_tensor)

    nc = get_nc(debug_taps=False)
    install_neuronx_cc_hook()
    shared = prep_shared(inputs)
    in_maps = [prep_core(inputs, shared, b) for b in range(B)]

    in_names, out_names, out_avals, zero_outs = [], [], [], []
    for alloc in nc.m.functions[0].allocations:
        if not isinstance(alloc, _mybir.MemoryLocationSet):
            continue
        name = alloc.memorylocations[0].name
        if alloc.kind == "ExternalInput":
            if nc.partition_id_tensor is None or name != nc.partition_id_tensor.name:
                in_names.append(name)
        elif alloc.kind == "ExternalOutput":
            shape = tuple(alloc.tensor_shape)
            dtype = _mybir.dt.np(alloc.dtype)
            out_names.append(name)
            out_avals.append(jax.core.ShapedArray(shape, dtype))
            zero_outs.append(np.zeros(shape, dtype))
    n_params = len(in_names)
    all_in = list(in_names) + list(out_names)
    if nc.partition_id_tensor is not None:
        all_in.append(nc.partition_id_tensor.name)

    def _body(*args):
        operands = list(args)
        if nc.partition_id_tensor is not None:
            operands.append(partition_id_tensor())
        return tuple(_bass_exec_p.bind(
            *operands, out_avals=tuple(out_avals), in_names=tuple(all_in),
            out_names=tuple(out_names), lowering_input_output_aliases=(),
            sim_require_finite=True, sim_require_nnan=True, nc=nc))

    devices = jax.devices()[:B]
    mesh = Mesh(np.asarray(devices), ("core",))
    spec = PartitionSpec("core")
    nin = n_params + len(zero_outs)
    sharded = jax.jit(
        shard_map(_body, mesh=mesh, in_specs=(spec,) * nin,
                  out_specs=(spec,) * len(out_names), check_rep=False),
        donate_argnums=tuple(range(n_params, nin)), keep_unused=True)

    concat_in = [np.concatenate([np.asarray(in_maps[c][n]) for c in range(B)], axis=0)
                 for n in in_names]
    sh = NamedSharding(mesh, spec)
    dev_in = [jax.device_put(a, sh) for a in concat_in]

    def one_call():
        zo = [np.zeros((B * z.shape[0], *z.shape[1:]), z.dtype) for z in zero_outs]
        t0 = time.perf_counter()
        out = sharded(*dev_in, *zo)
        jax.block_until_ready(out)
        return time.perf_counter() - t0, out

    one_call()  # warmup (compile)
    times = []
    out = None
    for _ in range(iters):
        dt, out = one_call()
        times.append(dt)
    loss_cat = np.asarray(out[out_names.index("loss")]).reshape(B, L)
    labels = np.asarray(inputs["labels"]).astype(np.int64)
    total = float(loss_cat.sum()) / np.count_nonzero(labels)
    return min(times), times, np.float32(total)



# revision 39
# speedup vs baseline: 1.0372x; 1.0372x over previous
"""DiarizeGPT Trainium2 kernel — data-parallel over batch (1 sample per NeuronCore, 8 cores).

Device dataflow per core (sample b):
  stage1: audio projection (PE, bf16) + packing (mask-mult + host-prepared dense add-in)
  8 decoder layers, canonical activation layout x[S=1152 (9 p-chunks), D=768] fp32 in SBUF:
    LN (bn_stats) -> h bf16 -> PE-transpose -> hT [768, 1152]
    qkv (bf16 weights), per-head causal attention with transposed scores [k, q],
    softmax denominator fused into the AV matmul via a ones-column on v (fp32r),
    attn_o + residual, MLP (fc -> exact gelu -> mlp_o) in q-blocks with PSUM-resident
    accumulation, + residual.
  final LN -> one-hot gather matmul (lat) -> head matmul -> log-softmax CE -> loss[90]
Host: shard/prep inputs (index/mask/one-hot artifacts + weight transposes, zero FLOPs),
 sum the 8x90 masked token losses and divide by the non-pad count.
"""

import numpy as np
import ml_dtypes

import concourse.bass as bass
import concourse.mybir as mybir
import concourse.tile as tile
from concourse import bacc
from concourse.bass_utils import run_bass_kernel_spmd

F32 = mybir.dt.float32
F32R = mybir.dt.float32r
BF16 = mybir.dt.bfloat16
AF = mybir.ActivationFunctionType
OP = mybir.AluOpType

P = 128
D = 768; H = 12; NL = 8; V = 128
LATENT = 1024; TA = 1000; L = 90; S = 1120; B = 8
SP = 1152          # padded sequence (9 * 128)
NSC = SP // P      # 9 sequence chunks
NDC = D // P       # 6 feature chunks
NLC = LATENT // P  # 8 latent chunks
DM = 4 * D         # 3072 mlp hidden
NMC = DM // P      # 24
DH = 64
NST = 3            # q strips of 384
ST = 384

_NC_CACHE = {}


def build_nc(debug_taps=False, n_layers=NL, sim_gelu_identity=False):
    nc = bacc.Bacc("TRN2", target_bir_lowering=False, debug=False)

    # ---- DRAM inputs (per-core data; weights identical across cores) ----
    d_audioT = nc.dram_tensor("audioT", [NLC, P, 1024], BF16, kind="ExternalInput")
    d_wp = nc.dram_tensor("wp", [NLC, P, D], BF16, kind="ExternalInput")
    d_addin = nc.dram_tensor("addin", [NSC, P, D], F32, kind="ExternalInput")
    d_maskA = nc.dram_tensor("maskA", [NSC, P], F32, kind="ExternalInput")
    d_wqk = nc.dram_tensor("wqk", [NL, NDC, P, 2 * D], BF16, kind="ExternalInput")
    d_wv = nc.dram_tensor("wv", [NL, NDC, P, D], BF16, kind="ExternalInput")
    d_wo = nc.dram_tensor("wo", [NL, NDC, P, D], BF16, kind="ExternalInput")
    d_fc = nc.dram_tensor("fc", [NL, NMC // 2, NDC, P, 2 * P], BF16, kind="ExternalInput")
    d_mo = nc.dram_tensor("mo", [NL, NMC, P, D], BF16, kind="ExternalInput")
    d_head = nc.dram_tensor("head", [NDC, P, V], F32, kind="ExternalInput")
    d_G = nc.dram_tensor("G", [NSC, P, L], F32, kind="ExternalInput")
    d_wsel = nc.dram_tensor("wsel", [L, V], F32, kind="ExternalInput")
    d_mrow = nc.dram_tensor("mrow", [L, 1], F32, kind="ExternalInput")
    d_loss = nc.dram_tensor("loss", [L, 1], F32, kind="ExternalOutput")

    taps = {}
    if debug_taps:
        taps["x0"] = nc.dram_tensor("tap_x0", [NSC, P, D], F32, kind="ExternalOutput")
        for l in range(n_layers):
            taps[f"x{l + 1}"] = nc.dram_tensor(f"tap_x{l + 1}", [NSC, P, D], F32,
                                               kind="ExternalOutput")
        taps["logits"] = nc.dram_tensor("tap_logits", [L, V], F32, kind="ExternalOutput")

    # inline constants (baked into NEFF, loaded at model-load time)
    ident_np = np.eye(P, dtype=ml_dtypes.bfloat16)
    d_ident = nc.inline_tensor(ident_np, name="identity")
    minv_np = np.zeros((P, 640), dtype=np.uint8)
    for r in range(P):
        minv_np[r, : min(512 + r, 640)] = 1.0  # 1 where INVALID (k > q)
    d_minv = nc.inline_tensor(minv_np, name="minv")

    with tile.TileContext(nc) as tc:
        with tc.tile_pool(name="singles", bufs=1) as singles, \
             tc.tile_pool(name="persist", bufs=1) as persist, \
             tc.tile_pool(name="wpool", bufs=1) as wpool, \
             tc.tile_pool(name="hT", bufs=1) as hTp, \
             tc.tile_pool(name="big", bufs=1) as bigp, \
             tc.tile_pool(name="str4", bufs=3) as str4, \
             tc.tile_pool(name="str3", bufs=3) as str3, \
             tc.tile_pool(name="denp", bufs=2) as denp, \
             tc.tile_pool(name="denrepp", bufs=2) as denrepp, \
             tc.tile_pool(name="sexpp", bufs=3) as sexpp, \
             tc.tile_pool(name="stats", bufs=6) as statsp, \
             tc.tile_pool(name="ps384", bufs=4, space="PSUM") as ps384, \
             tc.tile_pool(name="pout", bufs=2, space="PSUM") as poutp, \
             tc.tile_pool(name="dramp", bufs=2, space="DRAM") as dramp:

            # ---- singles ----
            ident_sb = singles.tile([P, P], BF16, tag="ident")
            nc.sync.dma_start(ident_sb, d_ident[:, :])
            minv_sb = singles.tile([P, 640], mybir.dt.uint8, tag="minv")
            nc.sync.dma_start(minv_sb, d_minv[:, :])
            zeros_sb = singles.tile([P, ST], BF16, tag="zeros")
            nc.vector.memset(zeros_sb, 0.0)
            eps_sb = singles.tile([P, 1], F32, tag="eps")
            nc.vector.memset(eps_sb, 1e-5)
            bias0_sb = singles.tile([P, 1], F32, tag="bias0")
            nc.vector.memset(bias0_sb, 0.0)
            maskA_sb = singles.tile([P, NSC], F32, tag="maskA")
            nc.sync.dma_start(maskA_sb, d_maskA[:, :].rearrange("c p -> p c"))
            G_sb = singles.tile([P, NSC, L], F32, tag="G")
            nc.sync.dma_start(G_sb, d_G[:, :, :].rearrange("c p j -> p c j"))
            head_sb = singles.tile([P, NDC, V], F32, tag="head")
            nc.sync.dma_start(head_sb, d_head[:, :, :].rearrange("c p v -> p c v"))
            wsel_sb = singles.tile([L, V], F32, tag="wsel")
            nc.sync.dma_start(wsel_sb, d_wsel[:, :])
            mrow_sb = singles.tile([L, 1], F32, tag="mrow")
            nc.sync.dma_start(mrow_sb, d_mrow[:, :])

            # persistent residual stream
            x_sb = persist.tile([P, NSC, D], F32, tag="x")

            # ================= stage 1: audio projection + packing =================
            with tc.tile_pool(name="stage1", bufs=1) as s1p, \
                 tc.tile_pool(name="addin", bufs=2) as adp, \
                 tc.tile_pool(name="s1str", bufs=3) as s1str:
                wp_sb = s1p.tile([P, NLC, D], BF16, tag="wp")
                nc.sync.dma_start(wp_sb, d_wp[:, :, :].rearrange("c p d -> p c d"))

                for sc in range(8):  # s rows 0..1023 get the audio matmul
                    at_sb = s1str.tile([P, NLC, P], BF16, tag="audioT")
                    nc.sync.dma_start(
                        at_sb, d_audioT[:, :, sc * P:(sc + 1) * P].rearrange("c p s -> p c s"))
                    ad = adp.tile([P, D], F32, tag="addin")
                    nc.sync.dma_start(ad, d_addin[sc, :, :])
                    for half in range(2):
                        cols = slice(half * ST, half * ST + ST)
                        pm = ps384.tile([P, ST], F32, tag="p384")
                        for lc in range(NLC):
                            nc.tensor.matmul(
                                pm, at_sb[:, lc, :],
                                wp_sb[:, lc, cols],
                                start=(lc == 0), stop=(lc == NLC - 1))
                        nc.vector.tensor_scalar_mul(
                            x_sb[:, sc, cols], pm, maskA_sb[:, sc:sc + 1])
                        nc.vector.tensor_add(
                            x_sb[:, sc, cols], x_sb[:, sc, cols], ad[:, cols])
                # pad chunk: pure add-in (start token / text can't reach past 1090)
                nc.sync.dma_start(x_sb[:, 8, :], d_addin[8, :, :])

            if debug_taps:
                nc.sync.dma_start(taps["x0"][:, :, :].rearrange("c p d -> p c d"), x_sb)

            # ================= decoder layers =================
            def layer_norm_to_hT(l, gi, hT_sb):
                """LN of x (free-dim stats) -> bf16 chunks -> PE transpose -> hT [p, dc, SP]."""
                for sc in range(NSC):
                    st_t = statsp.tile([P, 3, 6], F32, tag="bnst")
                    for g in range(3):
                        nc.vector.bn_stats(st_t[:, g, :], x_sb[:, sc, g * 256:(g + 1) * 256])
                    mv = statsp.tile([P, 2], F32, tag="bnmv")
                    nc.vector.bn_aggr(mv, st_t)
                    rstd = statsp.tile([P, 1], F32, tag="rstd")
                    nc.scalar.activation(rstd, mv[:, 1:2], AF.Sqrt, bias=eps_sb, scale=1.0)
                    nc.vector.reciprocal(rstd, rstd)
                    hc = str3.tile([P, D], BF16, tag="hc")
                    nc.vector.tensor_scalar(hc, x_sb[:, sc, :], mv[:, 0:1], rstd,
                                            OP.subtract, OP.mult)
                    for dc in range(NDC):
                        ptt = ps384.tile([P, ST], F32, tag="p384")
                        pt = ptt[:, :].bitcast(BF16)[:, 0:P]
                        nc.tensor.transpose(pt, hc[:, dc * P:(dc + 1) * P], ident_sb)
                        dst = hT_sb[:, dc, sc * P:(sc + 1) * P]
                        if dc % 2 == 0:
                            nc.vector.tensor_copy(dst, pt)
                        else:
                            nc.scalar.copy(dst, pt)

            for l in range(n_layers):
                # ---- layer weights (resident for the layer) ----
                wv_sb = wpool.tile([P, NDC, D], BF16, tag="wv")
                nc.sync.dma_start(wv_sb, d_wv[l].rearrange("c p n -> p c n"))
                wo_sb = wpool.tile([P, NDC, D], BF16, tag="wo")
                nc.sync.dma_start(wo_sb, d_wo[l].rearrange("c p n -> p c n"))

                # ---- LN1 + transpose ----
                hT_sb = hTp.tile([P, NDC, SP], BF16, tag="hT")
                layer_norm_to_hT(l, 0, hT_sb)

                # ---- q/k (transposed layout) ----
                qkT_sb = bigp.tile([P, H, SP], BF16, tag="qkT")
                for oc in range(H):  # 12 chunks of 128 rows: q = oc 0..5, k = oc 6..11
                    wqkb = str3.tile([P, NDC, P], BF16, tag="wqkb")
                    nc.sync.dma_start(
                        wqkb, d_wqk[l, :, :, oc * P:(oc + 1) * P].rearrange("c p n -> p c n"))
                    for ns in range(NST):
                        cols = slice(ns * ST, ns * ST + ST)
                        pm = ps384.tile([P, ST], F32, tag="p384")
                        for dc in range(NDC):
                            nc.tensor.matmul(
                                pm, wqkb[:, dc, :],
                                hT_sb[:, dc, cols],
                                start=(dc == 0), stop=(dc == NDC - 1))
                        nc.vector.tensor_copy(qkT_sb[:, oc, cols], pm)

                # ---- v (standard layout, 65-wide head blocks w/ ones column) ----
                v65_sb = bigp.tile([P, NSC, H, 65], BF16, tag="v65")
                for sc in range(NSC):
                    for half in range(2):
                        pm = ps384.tile([P, ST], F32, tag="p384")
                        for dc in range(NDC):
                            nc.tensor.matmul(
                                pm, hT_sb[:, dc, sc * P:(sc + 1) * P],
                                wv_sb[:, dc, half * ST:half * ST + ST],
                                start=(dc == 0), stop=(dc == NDC - 1))
                        nc.vector.tensor_copy(
                            v65_sb[:, sc, half * 6:(half + 1) * 6, 0:DH],
                            pm.rearrange("p (h e) -> p h e", e=DH))
                    nc.gpsimd.memset(v65_sb[:, sc, :, DH:65], 1.0)

                # ---- attention ----
                oT_sb = bigp.tile([P, NDC, SP], BF16, tag="oT")
                for hp in range(6):
                    den_h = [denp.tile([1, SP], F32, tag="denh", name=f"denh{_i}")
                             for _i in range(2)]
                    for st in range(NST):
                        K = 3 * (st + 1)
                        pav = [ps384.tile([P, ST], F32, tag="p384", name=f"pav{_i}")[0:65, :] for _i in range(2)]
                        se_prev = [None, None]
                        for ki in range(K):
                            o = ki * P - st * ST
                            q0 = max(0, o)
                            se_cur = [None, None]
                            for hh in range(2):
                                h = 2 * hp + hh
                                rows = slice(DH * hh, DH * hh + DH)
                                ps = ps384.tile([P, ST], F32, tag="p384")
                                nc.tensor.matmul(
                                    ps[:, q0:ST],
                                    qkT_sb[rows, 6 + hp, ki * P:(ki + 1) * P],
                                    qkT_sb[rows, hp, st * ST + q0:(st + 1) * ST],
                                    start=True, stop=True)
                                se = sexpp.tile([P, ST], BF16, tag="sexp")
                                nc.scalar.activation(se[:, q0:ST], ps[:, q0:ST],
                                                     AF.Exp, bias=bias0_sb, scale=0.125)
                                if o >= 0:
                                    nc.vector.copy_predicated(
                                        se[:, 0:o + P],
                                        minv_sb[:, 512 - o:640],
                                        zeros_sb[:, 0:o + P])
                                se_cur[hh] = se
                            # software pipeline: issue prev ki's AV after this ki's scores
                            for hh in range(2):
                                if ki > 0:
                                    nc.tensor.matmul(
                                        pav[hh],
                                        v65_sb[:, ki - 1, 2 * hp + hh, :],
                                        se_prev[hh][:, :],
                                        start=(ki == 1), stop=False)
                                se_prev[hh] = se_cur[hh]
                        for hh in range(2):
                            nc.tensor.matmul(
                                pav[hh],
                                v65_sb[:, K - 1, 2 * hp + hh, :],
                                se_prev[hh][:, :],
                                start=(K == 1), stop=True)
                        for hh in range(2):
                            h = 2 * hp + hh
                            cols = slice(st * ST, st * ST + ST)
                            nc.vector.tensor_copy(
                                oT_sb[DH * hh:DH * hh + DH, hp, cols], pav[hh][0:DH, :])
                            nc.scalar.copy(den_h[hh][0:1, cols], pav[hh][DH:DH + 1, :])
                    # normalize this head-pair's output chunk by 1/den
                    dden = dramp.tile([2, SP], BF16, tag="dden")
                    for hh in range(2):
                        denr = denp.tile([1, SP], F32, tag="denr")
                        nc.vector.reciprocal(denr, den_h[hh])
                        denrb = denp.tile([1, SP], BF16, tag="denrb")
                        nc.vector.tensor_copy(denrb, denr)
                        nc.sync.dma_start(dden[hh:hh + 1, :], denrb)
                    denrep = denrepp.tile([P, SP], BF16, tag="denrep")
                    for hh in range(2):
                        row = dden[hh:hh + 1, :]
                        bc = bass.AP(tensor=row.tensor, offset=row.offset,
                                     ap=[[0, DH], list(row.ap[1])])
                        nc.sync.dma_start(denrep[DH * hh:DH * hh + DH, :], bc)
                    nc.vector.tensor_tensor(oT_sb[:, hp, :], oT_sb[:, hp, :], denrep,
                                            OP.mult)

                # ---- attn_o + residual ----
                for sc in range(NSC):
                    for half in range(2):
                        cols = slice(half * ST, half * ST + ST)
                        pm = ps384.tile([P, ST], F32, tag="p384")
                        for dc in range(NDC):
                            nc.tensor.matmul(
                                pm, oT_sb[:, dc, sc * P:(sc + 1) * P],
                                wo_sb[:, dc, cols],
                                start=(dc == 0), stop=(dc == NDC - 1))
                        nc.vector.tensor_add(x_sb[:, sc, cols], x_sb[:, sc, cols], pm)

                # ---- LN2 + transpose ----
                h2T_sb = hTp.tile([P, NDC, SP], BF16, tag="hT")
                layer_norm_to_hT(l, 1, h2T_sb)

                # ---- MLP: fc -> gelu -> mlp_o, q-blocks of 2 chunks ----
                for qb0, qbn in ((0, 2), (2, 2), (4, 2), (6, 2), (8, 1)):
                    qcols = slice(qb0 * P, (qb0 + qbn) * P)
                    qw = qbn * P
                    pA = [poutp.tile([P, 512], F32, tag="poutA", name=f"pA{_i}")
                          for _i in range(qbn)]
                    pB = [poutp.tile([P, 256], F32, tag="poutB", name=f"pB{_i}")
                          for _i in range(qbn)]
                    hid_prev = None
                    for dm in range(NMC):
                        if dm % 2 == 0:
                            fcb_cur = str4.tile([P, NDC, 2 * P], BF16, tag="fcb")
                            nc.sync.dma_start(fcb_cur,
                                              d_fc[l, dm // 2].rearrange("c p n -> p c n"))
                        mob = str4.tile([P, D], BF16, tag="mob")
                        nc.sync.dma_start(mob, d_mo[l, dm])
                        ph = ps384.tile([P, ST], F32, tag="p384")
                        koff = (dm % 2) * P
                        for dc in range(NDC):
                            nc.tensor.matmul(
                                ph[:, 0:qw], fcb_cur[:, dc, koff:koff + P],
                                h2T_sb[:, dc, qcols],
                                start=(dc == 0), stop=(dc == NDC - 1))
                        hid = str3.tile([P, ST], BF16, tag="hid")
                        nc.scalar.activation(hid[:, 0:qw], ph[:, 0:qw],
                                             AF.Identity if sim_gelu_identity else AF.Gelu,
                                             bias=bias0_sb)
                        # pipeline: previous dm's mlp_o after this dm's fc
                        if hid_prev is not None:
                            for qc in range(qbn):
                                lh = hid_prev[0][:, qc * P:(qc + 1) * P]
                                nc.tensor.matmul(pA[qc], lh, hid_prev[1][:, 0:512],
                                                 start=(dm == 1), stop=False)
                                nc.tensor.matmul(pB[qc], lh, hid_prev[1][:, 512:D],
                                                 start=(dm == 1), stop=False)
                        hid_prev = (hid, mob)
                    for qc in range(qbn):
                        lh = hid_prev[0][:, qc * P:(qc + 1) * P]
                        nc.tensor.matmul(pA[qc], lh, hid_prev[1][:, 0:512],
                                         start=False, stop=True)
                        nc.tensor.matmul(pB[qc], lh, hid_prev[1][:, 512:D],
                                         start=False, stop=True)
                    for qc in range(qbn):
                        sc = qb0 + qc
                        nc.vector.tensor_add(x_sb[:, sc, 0:512], x_sb[:, sc, 0:512], pA[qc])
                        nc.vector.tensor_add(x_sb[:, sc, 512:D], x_sb[:, sc, 512:D], pB[qc])

                if debug_taps:
                    nc.sync.dma_start(
                        taps[f"x{l + 1}"][:, :, :].rearrange("c p d -> p c d"), x_sb)

            # ================= final LN (in place) + gather + head + CE =================
            for sc in range(NSC):
                st_t = statsp.tile([P, 3, 6], F32, tag="bnst")
                for g in range(3):
                    nc.vector.bn_stats(st_t[:, g, :], x_sb[:, sc, g * 256:(g + 1) * 256])
                mv = statsp.tile([P, 2], F32, tag="bnmv")
                nc.vector.bn_aggr(mv, st_t)
                rstd = statsp.tile([P, 1], F32, tag="rstd")
                nc.scalar.activation(rstd, mv[:, 1:2], AF.Sqrt, bias=eps_sb, scale=1.0)
                nc.vector.reciprocal(rstd, rstd)
                nc.vector.tensor_scalar(x_sb[:, sc, :], x_sb[:, sc, :], mv[:, 0:1], rstd,
                                        OP.subtract, OP.mult)

            latT_sb = singles.tile([P, NDC, L], F32, tag="latT")
            for dc in range(NDC):
                pl = ps384.tile([P, ST], F32, tag="p384", name="plat")[:, 0:L]
                for sc in range(NSC):
                    nc.tensor.matmul(
                        pl, x_sb[:, sc, dc * P:(dc + 1) * P],
                        G_sb[:, sc, :],
                        start=(sc == 0), stop=(sc == NSC - 1))
                nc.vector.tensor_copy(latT_sb[:, dc, :], pl)

            plog = ps384.tile([P, ST], F32, tag="p384", name="plog")[0:L, 0:V]
            for dc in range(NDC):
                nc.tensor.matmul(plog, latT_sb[:, dc, :],
                                 head_sb[:, dc, :],
                                 start=(dc == 0), stop=(dc == NDC - 1))
            lg = singles.tile([L, V], F32, tag="lg")
            nc.vector.tensor_copy(lg, plog)
            if debug_taps:
                nc.sync.dma_start(taps["logits"][:, :], lg)

            m1 = singles.tile([L, 1], F32, tag="m1")
            nc.vector.reduce_max(m1, lg, axis=mybir.AxisListType.X)
            negm = singles.tile([L, 1], F32, tag="negm")
            nc.vector.tensor_scalar_mul(negm, m1, -1.0)
            e_sb = singles.tile([L, V], F32, tag="e")
            s1 = singles.tile([L, 1], F32, tag="s1")
            nc.scalar.activation(e_sb, lg, AF.Exp, bias=negm, scale=1.0, accum_out=s1)
            lse = singles.tile([L, 1], F32, tag="lse")
            nc.scalar.activation(lse, s1, AF.Ln, bias=bias0_sb[0:L, :])
            junk = singles.tile([L, V], F32, tag="junk")
            tgt = singles.tile([L, 1], F32, tag="tgt")
            nc.vector.tensor_tensor(junk, lg, wsel_sb, OP.mult)
            nc.vector.reduce_sum(tgt, junk, axis=mybir.AxisListType.X)
            t1 = singles.tile([L, 1], F32, tag="t1")
            nc.vector.tensor_add(t1, m1, lse)
            nc.vector.tensor_tensor(t1, t1, tgt, OP.subtract)
            nc.vector.tensor_tensor(t1, t1, mrow_sb, OP.mult)
            nc.sync.dma_start(d_loss[:, :], t1)

    nc.compile()
    return nc


# ======================= host side =======================

def _sinu_pos(n, dim):
    half = dim // 2
    inv_freq = np.float32(10000.0) ** (-(np.arange(half, dtype=np.float32) / np.float32(half)))
    ang = np.arange(n, dtype=np.float32)[:, None] * inv_freq[None, :]
    return np.concatenate([np.sin(ang), np.cos(ang)], axis=-1).astype(np.float32)


def prep_shared(inp):
    """Weight-layout prep, shared across cores."""
    f32 = lambda a: np.asarray(a, dtype=np.float32)
    qkv_w = f32(inp["qkv_w"]); fc_w = f32(inp["fc_w"])
    attn_o_w = f32(inp["attn_o_w"]); mlp_o_w = f32(inp["mlp_o_w"])
    ln1_g = f32(inp["ln1_g"]); ln2_g = f32(inp["ln2_g"]); lnf_g = f32(inp["lnf_g"])
    bf = ml_dtypes.bfloat16

    wqk = np.empty((NL, NDC, P, 2 * D), dtype=bf)
    wv = np.empty((NL, NDC, P, D), dtype=bf)
    wo = np.empty((NL, NDC, P, D), dtype=bf)
    fcb = np.empty((NL, NMC // 2, NDC, P, 2 * P), dtype=bf)
    mo = np.empty((NL, NMC, P, D), dtype=bf)
    for l in range(NL):
        wqk_eff = (qkv_w[l, :2 * D] * ln1_g[l][None, :]).T          # [768, 1536]
        wv_eff = (qkv_w[l, 2 * D:] * ln1_g[l][None, :]).T           # [768, 768]
        wqk[l] = wqk_eff.reshape(NDC, P, 2 * D).astype(bf)
        wv[l] = wv_eff.reshape(NDC, P, D).astype(bf)
        wo[l] = attn_o_w[l].T.reshape(NDC, P, D).astype(bf)
        fc_eff = (fc_w[l] * ln2_g[l][None, :]).T                    # [768, 3072]
        fcb[l] = fc_eff.reshape(NDC, P, NMC // 2, 2 * P).transpose(2, 0, 1, 3).astype(bf)
        mo[l] = mlp_o_w[l].T.reshape(NMC, P, D).astype(bf)

    head = (f32(inp["text_head_w"]) * lnf_g[None, :]).T.reshape(NDC, P, V).copy()
    wp = f32(inp["audio_proj_w"]).T.reshape(NLC, P, D).astype(bf)

    posb_a = np.zeros((1024, D), dtype=np.float32)
    posb_a[:TA] = _sinu_pos(TA, D) * np.float32(inp["audio_pos_scale"]) \
        + f32(inp["audio_proj_b"])[None, :]
    pos_t = _sinu_pos(L, D) * np.float32(inp["text_pos_scale"])
    return dict(wqk=wqk, wv=wv, wo=wo, fc=fcb, mo=mo, head=head, wp=wp,
                posb_a=posb_a, pos_t=pos_t)


def prep_core(inp, shared, b):
    f32 = lambda a: np.asarray(a, dtype=np.float32)
    bf = ml_dtypes.bfloat16
    labels = np.asarray(inp["labels"]).astype(np.int64)[b]          # [90]
    al = int(np.asarray(inp["audio_lengths"]).astype(np.int64)[b])
    ll = int(np.asarray(inp["label_lengths"]).astype(np.int64)[b])
    text_emb_w = f32(inp["text_emb_w"])
    start_emb = f32(inp["start_emb"])

    audioT = np.zeros((LATENT, 1024), dtype=bf)
    audioT[:, :TA] = f32(inp["audio"][b]).T.astype(bf)

    maskA = np.zeros(SP, dtype=np.float32)
    maskA[:al] = 1.0

    addin = np.zeros((SP, D), dtype=np.float32)
    addin[:1024] = shared["posb_a"] * maskA[:1024, None]
    addin[al] = start_emb
    ntxt = ll - 1  # text tokens used as input (shifted right by start)
    if ntxt > 0:
        addin[al + 1: al + 1 + ntxt] = text_emb_w[labels[:ntxt]] + shared["pos_t"][:ntxt]

    G = np.zeros((SP, L), dtype=np.float32)
    G[al + np.arange(L), np.arange(L)] = 1.0

    wsel = np.zeros((L, V), dtype=np.float32)
    valid = labels != 0
    wsel[np.arange(L)[valid], labels[valid]] = 1.0
    mrow = valid.astype(np.float32).reshape(L, 1)

    return {
        "audioT": audioT.reshape(NLC, P, 1024),
        "addin": addin.reshape(NSC, P, D),
        "maskA": maskA.reshape(NSC, P),
        "G": G.reshape(NSC, P, L),
        "wsel": wsel, "mrow": mrow,
        "wp": shared["wp"], "wqk": shared["wqk"], "wv": shared["wv"],
        "wo": shared["wo"], "fc": shared["fc"], "mo": shared["mo"],
        "head": shared["head"],
    }


def get_nc(debug_taps=False, n_layers=NL):
    key = (bool(debug_taps), n_layers)
    if key not in _NC_CACHE:
        _NC_CACHE[key] = build_nc(debug_taps=key[0], n_layers=n_layers)
    return _NC_CACHE[key]


def run_cores(inputs, debug_taps=False, trace=False, n_layers=NL):
    nc = get_nc(debug_taps=debug_taps, n_layers=n_layers)
    shared = prep_shared(inputs)
    in_maps = [prep_core(inputs, shared, b) for b in range(B)]
    res = run_bass_kernel_spmd(nc, in_maps, core_ids=list(range(B)), trace=trace)
    return res


def kernel(**inputs) -> np.ndarray:
    res = run_cores(inputs)
    labels = np.asarray(inputs["labels"]).astype(np.int64)
    count = np.count_nonzero(labels)
    total = sum(float(r["loss"].sum()) for r in res.results)
    return np.float32(total / count)


def bench(inputs, iters=5):
    """Steady-state device execution timing: stage inputs on-device once, then
    time repeated jitted executions (mirrors bass2jax.run_bass_via_pjrt)."""
    import time
    import jax
    from jax.sharding import Mesh, PartitionSpec, NamedSharding
    from jax.experimental.shard_map import shard_map
    from concourse import mybir as _mybir
    from concourse.bass2jax import (_bass_exec_p, install_neuronx_cc_hook,
                                    partition_id_tensor)

    nc = get_nc(debug_taps=False)
    install_neuronx_cc_hook()
    shared = prep_shared(inputs)
    in_maps = [prep_core(inputs, shared, b) for b in range(B)]

    in_names, out_names, out_avals, zero_outs = [], [], [], []
    for alloc in nc.m.functions[0].allocations:
        if not isinstance(alloc, _mybir.MemoryLocationSet):
            continue
        name = alloc.memorylocations[0].name
        if alloc.kind == "ExternalInput":
            if nc.partition_id_tensor is None or name != nc.partition_id_tensor.name:
                in_names.append(name)
        elif alloc.kind == "ExternalOutput":
            shape = tuple(alloc.tensor_shape)
            dtype = _mybir.dt.np(alloc.dtype)
            out_names.append(name)
            out_avals.append(jax.core.ShapedArray(shape, dtype))
            zero_outs.append(np.zeros(shape, dtype))
    n_params = len(in_names)
    all_in = list(in_names) + list(out_names)
    if nc.partition_id_tensor is not None:
        all_in.append(nc.partition_id_tensor.name)

    def _body(*args):
        operands = list(args)
        if nc.partition_id_tensor is not None:
            operands.append(partition_id_tensor())
        return tuple(_bass_exec_p.bind(
            *operands, out_avals=tuple(out_avals), in_names=tuple(all_in),
            out_names=tuple(out_names), lowering_input_output_aliases=(),
            sim_require_finite=True, sim_require_nnan=True, nc=nc))

    devices = jax.devices()[:B]
    mesh = Mesh(np.asarray(devices), ("core",))
    spec = PartitionSpec("core")
    nin = n_params + len(zero_outs)
    sharded = jax.jit(
        shard_map(_body, mesh=mesh, in_specs=(spec,) * nin,
                  out_specs=(spec,) * len(out_names), check_rep=False),
        donate_argnums=tuple(range(n_params, nin)), keep_unused=True)

    concat_in = [np.concatenate([np.asarray(in_maps[c][n]) for c in range(B)], axis=0)
                 for n in in_names]
    sh = NamedSharding(mesh, spec)
    dev_in = [jax.device_put(a, sh) for a in concat_in]

    def one_call():
        zo = [np.zeros((B * z.shape[0], *z.shape[1:]), z.dtype) for z in zero_outs]
        t0 = time.perf_counter()
        out = sharded(*dev_in, *zo)
        jax.block_until_ready(out)
        return time.perf_counter() - t0, out

    one_call()  # warmup (compile)
    times = []
    out = None
    for _ in range(iters):
        dt, out = one_call()
        times.append(dt)
    loss_cat = np.asarray(out[out_names.index("loss")]).reshape(B, L)
    labels = np.asarray(inputs["labels"]).astype(np.int64)
    total = float(loss_cat.sum()) / np.count_nonzero(labels)
    return min(times), times, np.float32(total)

